# revision 30
# baseline (speedup 1.0000x reference)
"""DGCNN segmentation (3x EdgeConv max-aggregation + MLP head) on 8 Trainium2 cores.

Sharding: nodes are split into 8 equal contiguous blocks (one per core); each
core owns all edges whose *destination* lies in its block, so the scatter-max
aggregation is core-local.  Per-layer premultiplied node tables are computed
data-parallel over nodes and AllGather'd so every core can gather any source
node's contribution.

Per layer l (C_in -> C -> C, PyG EdgeConv):
    m_e   = relu(u'[dst_e] + v[src_e]) @ We_l          (per edge)
    h_i   = max_{e: dst_e = i} m_e + bb_l   (0 if no edges)
  where v = h @ Wa_l[C_in:]  (row table, gathered per edge),
        u' = h @ (Wa_l[:C_in] - Wa_l[C_in:]) + ba_l  (dst side, local bcast).

Device pipeline per core:
  - v row-tables in HBM (bf16); per-edge transposed gather via
    gpsimd.dma_gather(transpose=True) puts channels on partitions.  Gathers
    round-robin over 4 SWDGE queues so all four Q7 core-pairs generate
    descriptors in parallel (single queue leaves 3/4 of gpsimd idle).
  - Layer 1 needs no gather at all: the host uploads x rows in edge-slot
    order (layout-only prep), the first linear runs on the PE per tile.
  - DVE adds u'[dst] (broadcast over each node's slot run) to the gathered v,
    scalar-engine relu, PE matmul with We, segmented max over each node's
    padded slot-block via reduce_max, bias-add on eviction.
  - Edges are pre-sorted by destination and padded so each node owns a
    fixed-width slot run inside a 512-slot tile (identical tile structure on
    all 8 cores; only index data differs - the program is pure SPMD).
"""

import os
from dataclasses import dataclass, field

import numpy as np

import concourse.bass as bass
import concourse.mybir as mybir
import concourse.bacc as bacc
import concourse.tile as tile
from concourse import bass_utils, bass2jax
from concourse.bass import ds

F32 = mybir.dt.float32
BF16 = mybir.dt.bfloat16
I16 = mybir.dt.int16

NCORES = 8
TSLOT = 512          # edge-slots per tile (== max matmul moving free dim)
SUPER = 4            # tiles per dma_gather call
NQ = 4               # SWDGE queues used round-robin
AGP = 4              # AllGather pieces (table rows are piece-major)
EDGE_MAJOR = False   # slot order inside a tile: [d, n_t] vs [n_t, d]


# ----------------------------------------------------------------------------
# host-side preprocessing
# ----------------------------------------------------------------------------

@dataclass
class Plan:
    n: int
    npc: int
    tiles: list  # list of (D, n_t, pos0)  shared by all cores
    S: int       # total slots = TSLOT * len(tiles)
    perm: np.ndarray      # new position -> old node id
    vidx: list = field(default_factory=list)   # per-core wrapped [128, S/16] i16
    vsrc: list = field(default_factory=list)   # per-core flat [S] i64 (sentinel n)
    has_iso: bool = False  # any zero-degree node anywhere
    rpa: int = 0                               # local rows per AG piece
    psizes: list = field(default_factory=list)  # AG piece local row counts
    pbase: list = field(default_factory=list)   # piece base row in table


def make_plan(n: int, edge_index: np.ndarray) -> Plan:
    assert n % NCORES == 0
    npc = n // NCORES
    src = np.asarray(edge_index[0], dtype=np.int64)
    dst = np.asarray(edge_index[1], dtype=np.int64)
    deg = np.bincount(dst, minlength=n)

    # per-core block, degree-sorted (desc) within block
    perm = np.concatenate(
        [c * npc + np.argsort(-deg[c * npc:(c + 1) * npc], kind="stable")
         for c in range(NCORES)]
    )
    inv = np.empty(n, np.int64)
    inv[perm] = np.arange(n)
    src_n = inv[src]
    dst_n = inv[dst]
    deg_n = deg[perm]

    # shared tile structure from the max degree profile across cores
    degm = deg_n.reshape(NCORES, npc)
    maxdeg = degm.max(axis=0)
    tiles = []
    pos = 0
    while pos < npc:
        d = int(maxdeg[pos])
        d = max(2, d + (d & 1))          # even, >= 2
        n_t = min(TSLOT // d, npc - pos)
        tiles.append((d, n_t, pos))
        pos += n_t
    S = TSLOT * len(tiles)

    plan = Plan(n=n, npc=npc, tiles=tiles, S=S, perm=perm)
    plan.has_iso = bool((deg == 0).any())

    # AllGather piece geometry: tables are PIECE-MAJOR so each AG piece's
    # output is a contiguous slab: table row of node (c, r) with r in piece p
    # = pbase[p] + c*psizes[p] + (r - p*rpa);  sentinel row stays at n.
    rpa = min(npc, (npc * 55 // 100 + 127) // 128 * 128)
    psizes = []
    r = 0
    while r < npc:
        psizes.append(min(rpa, npc - r))
        r += rpa
    pbase = [int(v) for v in np.concatenate([[0], np.cumsum(
        [NCORES * s for s in psizes])[:-1]])]
    plan.rpa, plan.psizes, plan.pbase = rpa, psizes, pbase

    def remap(idx):
        out = np.full_like(idx, n)
        real = idx < n
        c, r = idx[real] // npc, idx[real] % npc
        p = np.minimum(r // rpa, len(psizes) - 1)
        pb = np.asarray(pbase)[p]
        sz = np.asarray(psizes)[p]
        out[real] = pb + c * sz + (r - p * rpa)
        return out

    # per-core slot fill.  L2/L3 tiles are EDGE-MAJOR inside each tile:
    # slot(node nti, edge di) = tile_base + di * n_t + nti.  This makes the
    # broadcast u'-add 2x-mode eligible on DVE (innermost dim is the packed
    # node dim for both operands); the edge matmul reads back node-major
    # via a strided AP so the segmented reduce stays innermost-axis.
    order = np.argsort(dst_n, kind="stable")
    src_s = src_n[order]
    dst_s = dst_n[order]
    starts = np.searchsorted(dst_s, np.arange(n))       # per new-id start
    for c in range(NCORES):
        dloc = deg_n[c * npc:(c + 1) * npc]
        vfill = np.full(npc, n, np.int64)      # sentinel: zero row
        nz = dloc > 0
        gids = c * npc + np.arange(npc)
        vfill[nz] = src_s[starts[gids[nz]]]    # first in-edge's src

        vidx = np.full(S, n, np.int64)
        base_pos = np.empty(npc, np.int64)   # slot of edge-rank 0 per node
        stride_pos = np.empty(npc, np.int64)  # slot stride between ranks
        for ti, (d, n_t, pos0) in enumerate(tiles):
            sl0 = ti * TSLOT
            p = np.arange(pos0, pos0 + n_t)
            if EDGE_MAJOR:
                base_pos[p] = sl0 + (p - pos0)
                stride_pos[p] = n_t
                vidx[sl0:sl0 + n_t * d] = np.tile(vfill[p], d)
            else:
                base_pos[p] = sl0 + (p - pos0) * d
                stride_pos[p] = 1
                vidx[sl0:sl0 + n_t * d] = np.repeat(vfill[p], d)
        # overwrite real edges
        m = (dst_s >= c * npc) & (dst_s < (c + 1) * npc)
        es, ed = src_s[m], dst_s[m] - c * npc
        # rank within node: edges of a node are contiguous since sorted by dst
        rank = np.arange(len(ed)) - np.searchsorted(ed, ed)
        slots = base_pos[ed] + rank * stride_pos[ed]
        vidx[slots] = es

        w = remap(vidx).astype(np.int16).reshape(-1, 16).T   # [16, S/16]
        plan.vidx.append(np.tile(w, (8, 1)).copy())   # [128, S/16]
        plan.vsrc.append(vidx)
    return plan


def prep_inputs(inputs: dict, plan: Plan) -> list:
    """Build per-core in_maps (keys = dram tensor names)."""
    n, npc, perm = plan.n, plan.npc, plan.perm
    f32 = np.float32
    import ml_dtypes
    bf16 = ml_dtypes.bfloat16

    x = np.asarray(inputs["x"], f32)[perm]              # [n, 3] permuted
    deg = np.bincount(np.asarray(inputs["edge_index"][1]), minlength=n)
    mask = (deg[perm] > 0).astype(f32)                  # new order

    w1a = np.asarray(inputs["w1a"], f32); b1a = np.asarray(inputs["b1a"], f32)
    w1b = np.asarray(inputs["w1b"], f32); b1b = np.asarray(inputs["b1b"], f32)
    w2a = np.asarray(inputs["w2a"], f32); b2a = np.asarray(inputs["b2a"], f32)
    w2b = np.asarray(inputs["w2b"], f32); b2b = np.asarray(inputs["b2b"], f32)
    w3a = np.asarray(inputs["w3a"], f32); b3a = np.asarray(inputs["b3a"], f32)
    w3b = np.asarray(inputs["w3b"], f32); b3b = np.asarray(inputs["b3b"], f32)
    wm1 = np.asarray(inputs["wm1"], f32); bm1 = np.asarray(inputs["bm1"], f32)
    wm2 = np.asarray(inputs["wm2"], f32); bm2 = np.asarray(inputs["bm2"], f32)
    wm3 = np.asarray(inputs["wm3"], f32); bm3 = np.asarray(inputs["bm3"], f32)

    # per-layer split: Wd = Wa[:cin]-Wa[cin:] (dst side), Wb = Wa[cin:] (src)
    w1a6 = np.zeros((8, 64), f32); w1a6[:6] = w1a       # [8, 64] K-pad
    wd2 = w2a[:64] - w2a[64:]                           # [64, 128]
    wb2 = w2a[64:]                                      # [64, 128]
    wd3 = w3a[:128] - w3a[128:]                         # [128, 256]
    wb3 = w3a[128:]                                     # [128, 256]

    # bb-fold: hstack holds h0 = h - bb (the segmented max without the
    # post-aggregation bias), so the reduce can write hstack directly.
    # Since everything downstream of h is linear in h, bb folds into the
    # consumers' biases:  pre_{l+1} = [h_i, h_j - h_i] @ Wa + ba
    #   = h0_i@Wd + h0_j@Wb + (ba + bb@Wa[:cin]);  head: bm1 += bbrow@wm1.
    # Only valid when no node is isolated (true h = h0 + bb for every node).
    if not plan.has_iso:
        b2a = b2a + b1b @ w2a[:64]
        b3a = b3a + b2b @ w3a[:128]
        bbrow = np.zeros(512, f32)
        bbrow[0:64] = b1b
        bbrow[128:256] = b2b
        bbrow[256:512] = b3b

    # second-linear (edge matmul) weights, bf16 lhsT layout
    we1 = w1b.astype(f32)                               # [64, 64]
    we2 = w2b.astype(f32)                               # [128, 128]
    we3 = np.ascontiguousarray(
        w3b.reshape(2, 128, 256).transpose(1, 0, 2))    # [128, k, 256]

    ba1 = b1a.reshape(64, 1).astype(f32)
    ba2 = b2a.reshape(128, 1).astype(f32)
    ba3 = b3a.reshape(2, 128).T.astype(f32)             # [128, 2]
    # post-aggregation bias (applied on eviction; bbm variant masks isolated)
    bb1 = b1b.reshape(64, 1).astype(f32)
    bb2 = b2b.reshape(128, 1).astype(f32)
    bb3 = b3b.reshape(2, 128).T.astype(f32)             # [128, 2]

    # bmask_l [C, J, npc] = bb * mask  (only used when isolated nodes exist)
    def bmask(bb_cj, mloc):
        # bb_cj: [C, J] f32 -> [C, J, npc] bf16 = bb * mask
        import ml_dtypes
        out = bb_cj[:, :, None] * mloc[None, None, :]
        return np.ascontiguousarray(out, dtype=ml_dtypes.bfloat16)

    # head weights: rearrange wm1 rows to hstack layout [h1(64) 0(64) h2 h3]
    wm1_arr = np.zeros((512, 512), f32)
    wm1_arr[0:64] = wm1[0:64]
    wm1_arr[128:256] = wm1[64:192]
    wm1_arr[256:512] = wm1[192:448]
    if not plan.has_iso:
        bm1 = bm1 + bbrow @ wm1_arr

    # edge features [x_dst, x_src - x_dst] in slot order (channel-major,
    # 8-padded); raw-input prep only, no model weights involved
    xpad = np.zeros((n + 1, 3), f32)
    xpad[:n] = x

    # per-slot dst node ids (from the fixed tile structure)
    dstid = np.empty(plan.S, np.int64)
    for ti, (d, n_t, pos0) in enumerate(plan.tiles):
        sl0 = ti * TSLOT
        p = np.arange(pos0, pos0 + n_t)
        dstid[sl0:sl0 + n_t * d] = (np.tile(p, d) if EDGE_MAJOR
                                    else np.repeat(p, d))
        dstid[sl0 + n_t * d:sl0 + TSLOT] = pos0

    in_maps = []
    for c in range(NCORES):
        mloc = mask[c * npc:(c + 1) * npc]
        xs = xpad[plan.vsrc[c]]                             # [S, 3] src rows
        xd = x[c * npc:(c + 1) * npc][dstid]                # [S, 3] dst rows
        xe = np.zeros((8, plan.S), f32)
        xe[0:3] = xd.T
        xe[3:6] = (xs - xd).T
        # sentinel slots (isolated nodes): zero feature
        sent = plan.vsrc[c] >= n
        xe[:, sent] = 0.0
        m = {
            "xe": np.ascontiguousarray(xe).astype(bf16),
            "vidx": plan.vidx[c],
            "w1a6": w1a6.astype(bf16),
            "wd2": wd2.astype(bf16), "wd3": wd3.astype(bf16),
            "wb2": wb2.astype(bf16), "wb3": wb3.astype(bf16),
            "we1": we1.astype(bf16), "we2": we2.astype(bf16),
            "we3": we3.astype(bf16),
            "ba1": ba1, "ba2": ba2, "ba3": ba3,
            "bb1": bb1, "bb2": bb2, "bb3": bb3,
            "bm1": bmask(bb1, mloc), "bm2": bmask(bb2, mloc),
            "bm3": bmask(bb3, mloc),
            "wh1": np.ascontiguousarray(
                wm1_arr.reshape(4, 128, 512).transpose(1, 0, 2)).astype(bf16),
            "wh2": np.ascontiguousarray(
                wm2.reshape(4, 128, 256).transpose(1, 0, 2)).astype(bf16),
            "wh3": np.ascontiguousarray(
                wm3.reshape(2, 128, 4).transpose(1, 0, 2)).astype(bf16),
            "bh1": np.ascontiguousarray(bm1.reshape(4, 128).T),
            "bh2": np.ascontiguousarray(bm2.reshape(2, 128).T),
            "bh3": bm3.reshape(4, 1).astype(f32),
        }
        in_maps.append(m)
    return in_maps


# ----------------------------------------------------------------------------
# device program
# ----------------------------------------------------------------------------

LAYERS = [
    # cin: input chans, cmid: relu width, cout: out chans, J: 128-chunks of
    # cmid/cout, src_chunk: hstack chunk range of the input
    dict(name="1", cin=3, cmid=64, cout=64, J=1, out_chunks=[0]),
    dict(name="2", cin=64, cmid=128, cout=128, J=1, out_chunks=[1]),
    dict(name="3", cin=128, cmid=256, cout=256, J=2, out_chunks=[2, 3]),
]


def build_program(plan: Plan, nlayers: int = 3, with_head: bool = True,
                  with_edge: bool = True, reps: int = 1):
    assert reps == 1 or (nlayers == 3 and with_head and with_edge)
    n, npc, S = plan.n, plan.npc, plan.S
    ntiles = len(plan.tiles)
    nc = bacc.Bacc(
        "TRN2", target_bir_lowering=False, debug=False,
        enable_asserts=False, num_devices=NCORES, num_swdge_queues=NQ,
    )
    RG = [list(range(NCORES))]

    # ---- dram tensors -------------------------------------------------------
    din = {}
    def dram_in(name, shape, dt):
        din[name] = nc.dram_tensor(name, list(shape), dt, kind="ExternalInput")
        return din[name]

    xe_d = dram_in("xe", (8, S), BF16)
    vidx_d = dram_in("vidx", (128, S // 16), I16)
    w1a6_d = dram_in("w1a6", (8, 64), BF16)
    wd_d = [None, dram_in("wd2", (64, 128), BF16),
            dram_in("wd3", (128, 256), BF16)]
    wb_d = [None, dram_in("wb2", (64, 128), BF16),
            dram_in("wb3", (128, 256), BF16)]
    we_d = [dram_in("we1", (64, 64), BF16), dram_in("we2", (128, 128), BF16),
            dram_in("we3", (128, 2, 256), BF16)]
    ba_d = [dram_in("ba1", (64, 1), F32), dram_in("ba2", (128, 1), F32),
            dram_in("ba3", (128, 2), F32)]
    bb_d = [dram_in("bb1", (64, 1), F32), dram_in("bb2", (128, 1), F32),
            dram_in("bb3", (128, 2), F32)]
    if plan.has_iso:
        bm_d = [dram_in("bm1", (64, 1, npc), BF16),
                dram_in("bm2", (128, 1, npc), BF16),
                dram_in("bm3", (128, 2, npc), BF16)]
    wh_d = [dram_in("wh1", (128, 4, 512), BF16),
            dram_in("wh2", (128, 4, 256), BF16),
            dram_in("wh3", (128, 2, 4), BF16)]
    bh_d = [dram_in("bh1", (128, 4), F32), dram_in("bh2", (128, 2), F32),
            dram_in("bh3", (4, 1), F32)]
    outT = nc.dram_tensor("outT", [4, npc], F32, kind="ExternalOutput")

    # Tables for layers 2,3.  The AllGather moves raw h (cin channels,
    # channel-major slabs — half the bytes of premultiplied v); each core
    # then computes v = h @ Wb for ALL nodes locally and writes the
    # row-major gather table vtab.  dma_gather cannot read Shared-addr-
    # space scratchpad, so everything stays Local.
    vcols = [None, 128, 256]          # v width per consuming layer
    hcols = [None, 64, 128]           # h (AG payload) width
    nbuf = min(2, reps)               # table double-buffering by rep parity
    vtab_t = [[None] + [nc.dram_tensor(f"vtab{i}_{b}",
                                       [n + 1, vcols[i - 1]],
                                       BF16, kind="Internal")
                        for i in (2, 3)] for b in range(nbuf)]
    hagp_t = [[None] for _ in range(nbuf)]
    tabTp_t = [[None] for _ in range(nbuf)]
    for b in range(nbuf):
        for i in (2, 3):
            hagp_t[b].append([nc.dram_tensor(
                f"hag{i}_{p}_{b}", [hcols[i - 1], plan.psizes[p]], BF16,
                kind="Internal") for p in range(len(plan.psizes))])
            tabTp_t[b].append([nc.dram_tensor(
                f"tabT{i}_{p}_{b}",
                [NCORES * hcols[i - 1], plan.psizes[p]], BF16,
                kind="Internal") for p in range(len(plan.psizes))])

    with tile.TileContext(nc) as tc:
        with (
            tc.tile_pool(name="singles", bufs=1) as sing,
            tc.tile_pool(name="stage", bufs=3) as stg,
            tc.tile_pool(name="xe", bufs=2) as xep,
            tc.tile_pool(name="gat2", bufs=3) as gat2,
            tc.tile_pool(name="gat3", bufs=4) as gat3,
            tc.tile_pool(name="edge", bufs=3) as edg,
            tc.tile_pool(name="vtj2", bufs=2) as vtj2,
            tc.tile_pool(name="vtj3", bufs=2) as vtj3,
            tc.tile_pool(name="hst", bufs=2) as hsp,
            tc.tile_pool(name="us", bufs=1) as usp,
        ):
            vtj = [None, vtj2, vtj3]
            # ---- load constants into SBUF ----
            def load(dt_handle, shape, dtype, tag):
                t = sing.tile(list(shape), dtype, tag=tag)
                nc.sync.dma_start(t, dt_handle[...])
                return t

            vidx_s = load(vidx_d, (128, S // 16), I16, "vidx")
            w1a6_s = load(w1a6_d, (8, 64), BF16, "w1a6")
            wd_s = [None] + [load(wd_d[i], wd_d[i].shape, BF16, f"wd{i}")
                             for i in (1, 2)]
            wb_s = [None] + [load(wb_d[i], wb_d[i].shape, BF16, f"wb{i}")
                             for i in (1, 2)]
            we_s = [load(we_d[i], we_d[i].shape, BF16, f"we{i}")
                    for i in range(3)]
            ba_s = [load(ba_d[i], ba_d[i].shape, F32, f"ba{i}")
                    for i in range(3)]
            bb_s = [load(bb_d[i], bb_d[i].shape, F32, f"bb{i}")
                    for i in range(3)]
            bm_s = ([load(bm_d[i], bm_d[i].shape, BF16, f"bm{i}")
                     for i in range(3)] if plan.has_iso else None)
            wh_s = [load(wh_d[i], wh_d[i].shape, BF16, f"wh{i}")
                    for i in range(3)]
            bh_s = [load(bh_d[i], bh_d[i].shape, F32, f"bh{i}")
                    for i in range(3)]

            zrow = sing.tile([1, 512], BF16)
            nc.vector.memset(zrow, 0.0)

            # hstack [h1;0 | h2 | h3a | h3b], u' tables and the output
            # staging buffer are double-buffered by rep parity so rep i+1's
            # L1 phase can overlap rep i's head (WAR would otherwise
            # serialize the unrolled benchmark pipeline)
            hstack = None
            u_s = None

            # ---- table builders -------------------------------------------
            # Layer l's u'/v tables are built from layer l-1's output, whose
            # node aggregations complete progressively (tiles are position-
            # ordered).  Builders are *emitted* interleaved with the producing
            # layer's edge loop so the PE executes them as soon as the data is
            # ready, and the AllGather runs in AGP staggered pieces that
            # overlap the edge phase instead of serializing after it.
            stagger = npc // 16

            def hsrc(li):
                return hstack[0:64, 0, :] if li == 1 else hstack[:, 1, :]

            def u_builder(li, pool):
                L = LAYERS[li]
                cpj = L["cmid"] // L["J"]
                J = L["J"]
                hprev = hsrc(li)
                wdl = wd_s[li]
                state = dict(c=0)
                UW = 256

                def emit(ready):
                    while state["c"] < npc:
                        c0 = state["c"]
                        w = min(UW, npc - c0)
                        if c0 + w > ready:
                            return
                        pu = pool.tile([128, 2, UW], F32, tag="ub",
                                       name="pu", bufs=1)
                        for jj in range(J):
                            nc.tensor.matmul(pu[0:cpj, jj, 0:w],
                                             wdl[:, ds(jj * 128, cpj)],
                                             hprev[:, ds(c0, w)],
                                             start=True, stop=True)
                            nc.scalar.activation(
                                u_s[li][0:cpj, jj, ds(c0, w)],
                                pu[0:cpj, jj, 0:w],
                                mybir.ActivationFunctionType.Identity,
                                bias=ba_s[li][:, jj:jj + 1])
                        state["c"] = c0 + w
                return emit

            def v_builder(li, pool, tb):
                cv = LAYERS[li]["cmid"]          # v table width (== cout)
                cin = LAYERS[li]["cin"]          # AG payload channels
                kch = 512 // cv                  # node chunks per PSUM bank
                hprev = hsrc(li)
                tjp = vtj[li]
                vtab = vtab_t[tb][li]
                hagp = hagp_t[tb][li]
                tabTp = tabTp_t[tb][li]
                state = dict(a=0)
                rpa = plan.rpa
                nc.sync.dma_start(vtab[n:n + 1, :], zrow[:, 0:cv])

                def emit(ready, flush=False):
                    while state["a"] < len(plan.psizes):
                        p = state["a"]
                        a0 = p * rpa
                        rows = plan.psizes[p]
                        if a0 + rows > ready:
                            break
                        if not flush and a0 + rows + stagger > ready:
                            break
                        # stage local h slab (channel-major) and AllGather
                        nc.sync.dma_start(hagp[p][...],
                                          hprev[0:cin, ds(a0, rows)])
                        nc.gpsimd.collective_compute(
                            "AllGather", mybir.AluOpType.bypass, RG,
                            ins=[hagp[p][...]],
                            outs=[tabTp[p][...]],
                        )
                        # v rows for every core's block of this piece
                        for c in range(NCORES):
                            # per-core sub-slab of the gathered table
                            tj = tjp.tile([128, rpa], BF16,
                                          tag=f"tj{li}", name="tj")
                            nc.sync.dma_start(
                                tj[0:cin, 0:rows],
                                tabTp[p][ds(c * cin, cin), :])
                            base = plan.pbase[p] + c * rows
                            j0 = 0
                            while j0 < rows:
                                g = min(kch * 128, rows - j0)
                                nk = (g + 127) // 128
                                pv = pool.tile([128, kch, cv], F32,
                                               tag="vrow", name="pv",
                                               bufs=1)
                                for k in range(nk):
                                    m = min(128, g - k * 128)
                                    nc.tensor.matmul(
                                        pv[0:m, k, 0:cv],
                                        tj[0:cin, ds(j0 + k * 128, m)],
                                        wb_s[li][...],
                                        start=True, stop=True)
                                st = stg.tile([128, kch, cv], BF16,
                                              tag="uv_stage")
                                eng = nc.scalar if c % 2 == 0 else nc.vector
                                if g == kch * 128:
                                    if eng is nc.scalar:
                                        eng.copy(st[...], pv[...])
                                    else:
                                        eng.tensor_copy(st[...], pv[...])
                                    nc.sync.dma_start(
                                        vtab[ds(base + j0, g), :]
                                        .rearrange("(k p) v -> p k v",
                                                   p=128),
                                        st[...])
                                else:
                                    for k in range(nk):
                                        m = min(128, g - k * 128)
                                        if eng is nc.scalar:
                                            eng.copy(st[0:m, k, 0:cv],
                                                     pv[0:m, k, 0:cv])
                                        else:
                                            eng.tensor_copy(
                                                st[0:m, k, 0:cv],
                                                pv[0:m, k, 0:cv])
                                        nc.sync.dma_start(
                                            vtab[
                                                ds(base + j0 + k * 128,
                                                   m), :],
                                            st[0:m, k, 0:cv])
                                j0 += g
                        state["a"] += 1
                return emit

            # ---- head builder (interleaved into L3's edge phase) ---------
            HW = 256

            def head_builder(pool):
                state = dict(c=0)

                def emit(ready, flush=False):
                    while state["c"] < npc:
                        c0 = state["c"]
                        w = min(HW, npc - c0)
                        if not flush and c0 + w + stagger > ready:
                            break
                        ps1 = pool.tile([128, 4, HW], F32, tag="h_ps1",
                                        name="ps1", bufs=1)
                        for jj in range(4):
                            for kk in range(4):
                                nc.tensor.matmul(
                                    ps1[:, jj, 0:w],
                                    wh_s[0][:, kk, ds(jj * 128, 128)],
                                    hstack[:, kk, ds(c0, w)],
                                    start=(kk == 0), stop=(kk == 3))
                        m1 = edg.tile([128, 4, HW], BF16, tag="h_m1")
                        for jj in range(4):
                            nc.scalar.activation(
                                m1[:, jj, 0:w], ps1[:, jj, 0:w],
                                mybir.ActivationFunctionType.Relu,
                                bias=bh_s[0][:, jj:jj + 1])
                        ps2 = pool.tile([128, 2, HW], F32, tag="h_ps2",
                                        name="ps2", bufs=1)
                        for jj in range(2):
                            for kk in range(4):
                                nc.tensor.matmul(
                                    ps2[:, jj, 0:w],
                                    wh_s[1][:, kk, ds(jj * 128, 128)],
                                    m1[:, kk, 0:w],
                                    start=(kk == 0), stop=(kk == 3))
                        m2 = edg.tile([128, 2, HW], BF16, tag="h_m2")
                        for jj in range(2):
                            nc.scalar.activation(
                                m2[:, jj, 0:w], ps2[:, jj, 0:w],
                                mybir.ActivationFunctionType.Relu,
                                bias=bh_s[1][:, jj:jj + 1])
                        ps3 = pool.tile([4, HW], F32, tag="h_ps3",
                                        name="ps3", bufs=1)
                        for kk in range(2):
                            nc.tensor.matmul(ps3[:, 0:w], wh_s[2][:, kk, :],
                                             m2[:, kk, 0:w],
                                             start=(kk == 0), stop=(kk == 1))
                        oc_t = stg.tile([4, HW], F32, tag="oc")
                        nc.scalar.activation(
                            oc_t[:, 0:w], ps3[:, 0:w],
                            mybir.ActivationFunctionType.Identity,
                            bias=bh_s[2][:, 0:1])
                        nc.sync.dma_start(outT[:, ds(c0, w)], oc_t[:, 0:w])
                        state["c"] = c0 + w
                return emit

            # =============== per layer ===============
            # reps > 1 unrolls the whole computation back-to-back for
            # device-time benchmarking (amortizes per-dispatch overhead)
            from contextlib import ExitStack
            for rep, (li, L) in ((r, x) for r in range(reps)
                                 for x in enumerate(LAYERS[:nlayers])):
                cin, cmid, cout, J = L["cin"], L["cmid"], L["cout"], L["J"]
                if li == 0:
                    hstack = hsp.tile([128, 4, npc], BF16, tag="hstack",
                                      name="hstack")
                    u_s = [None,
                           usp.tile([128, 1, npc], BF16, tag="u2",
                                    name="u2"),
                           usp.tile([128, 2, npc], BF16, tag="u3",
                                    name="u3")]
                    if rep < 2:
                        # rows 64:128 of chunk 0 are never written by L1
                        # but are read by the head (against zero weights);
                        # keep them finite.  Later reps reuse finite data.
                        nc.vector.memset(
                            hstack[64:128, 0, :] if with_edge
                            else hstack[...], 0.0)
                with ExitStack() as phase:
                    psp = phase.enter_context(tc.tile_pool(
                        name=f"ps{rep}_{li}", bufs=2, space="PSUM"))
                    # builders for the NEXT layer run inside this edge phase
                    nxt_u = (u_builder(li + 1, psp)
                             if li + 1 < nlayers else None)
                    nxt_v = (v_builder(li + 1, psp, rep % nbuf)
                             if li + 1 < nlayers else None)
                    head = (head_builder(psp)
                            if (with_head and li == nlayers - 1) else None)

                    # ---- edge phase ----
                    ebufs = 3 if li == 0 else (4 if li == 1 else 2)
                    for sti, st0 in enumerate(range(0, ntiles, SUPER)
                                              if with_edge else []):
                        g = min(SUPER, ntiles - st0)
                        nidx = g * TSLOT
                        if li == 0:
                            xet = xep.tile([8, SUPER * TSLOT], BF16, tag="xe")
                            nc.sync.dma_start(
                                xet[:, 0:nidx],
                                xe_d[:, ds(st0 * TSLOT, nidx)])
                        else:
                            pool = gat2 if li == 1 else gat3
                            vg = pool.tile([128, J, nidx], BF16,
                                           tag=f"vg{li}", name=f"vg{li}")
                            c0 = st0 * TSLOT // 16
                            nc.gpsimd.dma_gather(
                                vg[...], vtab_t[rep % nbuf][li][...],
                                vidx_s[:, ds(c0, nidx // 16)],
                                nidx, nidx, elem_size=cout, transpose=True,
                                single_packet=False, queue_num=sti % NQ)
                        for tt in range(g):
                            d, n_t, pos0 = plan.tiles[st0 + tt]
                            T = n_t * d
                            o = tt * TSLOT
                            if li == 0:
                                # pre = W1a^T [x_i, x_j-x_i]  (host-built xe)
                                pp = psp.tile([64, 1, TSLOT], F32,
                                              tag="pre_ps", name="pp", bufs=3)
                                nc.tensor.matmul(pp[:, 0, 0:T], w1a6_s,
                                                 xet[:, ds(o, T)],
                                                 start=True, stop=True)
                                rl = edg.tile([64, 1, TSLOT], BF16, tag="rl1")
                                nc.scalar.activation(
                                    rl[:, 0, 0:T], pp[:, 0, 0:T],
                                    mybir.ActivationFunctionType.Relu,
                                    bias=ba_s[0][:, 0:1])
                            else:
                                if EDGE_MAJOR:
                                    # slots [d, n_t], node dim innermost:
                                    # both add operands 2-byte packed (2x)
                                    ub = u_s[li][:, :, ds(pos0, n_t)
                                                 ].rearrange(
                                        "p j (one nt) -> p j one nt",
                                        one=1).broadcast_to(
                                        (128, J, d, n_t))
                                    vgv = vg[:, :, ds(o, T)].rearrange(
                                        "p j (d nt) -> p j d nt", nt=n_t)
                                else:
                                    ub = u_s[li][:, :, ds(pos0, n_t)
                                                 ].rearrange(
                                        "p j (nt one) -> p j nt one",
                                        one=1).broadcast_to(
                                        (128, J, n_t, d))
                                    vgv = vg[:, :, ds(o, T)].rearrange(
                                        "p j (nt d) -> p j nt d", d=d)
                                pre = edg.tile([128, J, TSLOT], BF16,
                                               tag=f"pre{li}", name="pre")
                                nc.vector.tensor_add(
                                    pre[:, :, 0:T].rearrange(
                                        "p j (a b) -> p j a b",
                                        b=n_t if EDGE_MAJOR else d),
                                    vgv, ub)
                                rl = edg.tile([128, J, TSLOT], BF16,
                                              tag=f"rl{li}")
                                nc.scalar.activation(
                                    rl[:, :, 0:T], pre[:, :, 0:T],
                                    mybir.ActivationFunctionType.Relu)
                            # m = rl @ We  (K-accumulate over J chunks).
                            # In edge-major mode the rhs reads rl back in
                            # node-major order so PSUM comes out node-major
                            # and the segmented max reduces innermost.
                            ps = psp.tile([128, J, TSLOT], F32,
                                          tag="edge_ps", name="ps",
                                          bufs=ebufs)
                            for jj in range(J):
                                for kk in range(J):
                                    w = (we_s[li][0:cmid, :] if J == 1
                                         else we_s[li][:, kk,
                                                       ds(jj * 128, 128)])
                                    rhs = rl[:, kk, 0:T]
                                    if EDGE_MAJOR:
                                        rhs = rhs.rearrange(
                                            "p (d nt) -> p nt d", nt=n_t)
                                    nc.tensor.matmul(
                                        ps[0:cout // J, jj, 0:T], w, rhs,
                                        start=(kk == 0), stop=(kk == J - 1))
                            oc = L["out_chunks"]
                            if plan.has_iso:
                                # segmented max + bias*mask add
                                tmp = edg.tile([128, 2, 256], BF16, tag="agg")
                                nc.vector.reduce_max(
                                    tmp[0:cout // J, 0:J, 0:n_t],
                                    ps[0:cout // J, 0:J, 0:T].rearrange(
                                        "p j (nt d) -> p j nt d", d=d),
                                    axis=mybir.AxisListType.X)
                                nc.vector.tensor_add(
                                    hstack[0:cout // J, oc[0]:oc[0] + J,
                                           ds(pos0, n_t)],
                                    tmp[0:cout // J, 0:J, 0:n_t],
                                    bm_s[li][:, :, ds(pos0, n_t)])
                            else:
                                # bb is folded downstream: segmented max
                                # writes hstack directly
                                nc.vector.reduce_max(
                                    hstack[0:cout // J, oc[0]:oc[0] + J,
                                           ds(pos0, n_t)],
                                    ps[0:cout // J, 0:J, 0:T].rearrange(
                                        "p j (nt d) -> p j nt d", d=d),
                                    axis=mybir.AxisListType.X)
                        # emit next-layer table chunks whose inputs are done
                        last = plan.tiles[st0 + g - 1]
                        ready = last[2] + last[1]
                        if nxt_u is not None:
                            nxt_u(ready)
                            nxt_v(ready)
                        if head is not None:
                            head(ready)
                    if nxt_u is not None:
                        nxt_u(npc)
                        nxt_v(npc, flush=True)
                    if head is not None:
                        head(npc, flush=True)

            # =============== head (debug fallback paths) ===============
            if not with_head:
                with tc.tile_pool(name="psum1", bufs=1, space="PSUM"):
                    dbg = sing.tile([4, npc], F32, tag="dbg")
                    nc.vector.tensor_copy(dbg, hstack[0:4, 0, :])
                    nc.sync.dma_start(outT[...], dbg)
            elif nlayers < 3:
                with tc.tile_pool(name="psum1", bufs=1, space="PSUM") as psp1:
                    hb = head_builder(psp1)
                    hb(npc, flush=True)

    nc.compile()
    return nc


# ----------------------------------------------------------------------------
# entry point
# ----------------------------------------------------------------------------

class Runner:
    """Compile once; keep the jitted sharded executable for cheap re-runs."""

    def __init__(self, nc):
        import jax
        from jax.sharding import Mesh, PartitionSpec
        from jax.experimental.shard_map import shard_map

        bass2jax.install_neuronx_cc_hook()
        self.nc = nc
        partition_name = (nc.partition_id_tensor.name
                          if nc.partition_id_tensor else None)
        in_names, out_names, out_avals, zero_outs = [], [], [], []
        for alloc in nc.m.functions[0].allocations:
            if not isinstance(alloc, mybir.MemoryLocationSet):
                continue
            name = alloc.memorylocations[0].name
            if alloc.kind == "ExternalInput":
                if name != partition_name:
                    in_names.append(name)
            elif alloc.kind == "ExternalOutput":
                shape = tuple(alloc.tensor_shape)
                dtype = mybir.dt.np(alloc.dtype)
                out_names.append(name)
                out_avals.append(jax.core.ShapedArray(shape, dtype))
                zero_outs.append(np.zeros(shape, dtype))
        n_params = len(in_names)
        all_in = in_names + out_names
        if partition_name is not None:
            all_in.append(partition_name)
        self.in_names = in_names
        self.out_names = out_names
        self.out_avals = out_avals
        self.zero_outs = zero_outs

        def _body(*args):
            operands = list(args)
            if partition_name is not None:
                operands.append(bass2jax.partition_id_tensor())
            return tuple(bass2jax._bass_exec_p.bind(
                *operands, out_avals=tuple(out_avals),
                in_names=tuple(all_in), out_names=tuple(out_names),
                lowering_input_output_aliases=(),
                sim_require_finite=True, sim_require_nnan=True, nc=nc))

        devices = jax.devices()[:NCORES]
        mesh = Mesh(np.asarray(devices), ("core",))
        self.mesh = mesh
        n_outs = len(out_names)
        self.sharded = jax.jit(
            shard_map(_body, mesh=mesh,
                      in_specs=(PartitionSpec("core"),) * (n_params + n_outs),
                      out_specs=(PartitionSpec("core"),) * n_outs,
                      check_rep=False),
            keep_unused=True)

    def prepare(self, in_maps):
        """Upload per-core inputs to the devices once; returns dev args."""
        import jax
        from jax.sharding import NamedSharding, PartitionSpec
        sh = NamedSharding(self.mesh, PartitionSpec("core"))
        concat_in = [
            np.concatenate([np.asarray(in_maps[c][nm])
                            for c in range(NCORES)], axis=0)
            for nm in self.in_names]
        concat_zeros = [np.zeros((NCORES * z.shape[0], *z.shape[1:]), z.dtype)
                        for z in self.zero_outs]
        args = [jax.device_put(a, sh) for a in concat_in + concat_zeros]
        jax.block_until_ready(args)
        return args

    def run(self, dev_args):
        import jax
        outs = self.sharded(*dev_args)
        jax.block_until_ready(outs)
        return outs

    def __call__(self, in_maps):
        outs = self.run(self.prepare(in_maps))
        return [
            {nm: np.asarray(outs[i]).reshape(
                NCORES, *self.out_avals[i].shape)[c]
             for i, nm in enumerate(self.out_names)}
            for c in range(NCORES)]


_CACHE = {}


def get_compiled(inputs: dict):
    x = np.asarray(inputs["x"])
    ei = np.asarray(inputs["edge_index"])
    n = x.shape[0]
    key = (n, ei.shape[1], hash(ei.tobytes()))
    if key not in _CACHE:
        plan = make_plan(n, ei)
        nc = build_program(plan)
        _CACHE.clear()
        _CACHE[key] = (plan, Runner(nc))
    return _CACHE[key]


def _run(inputs: dict) -> np.ndarray:
    plan, runner = get_compiled(inputs)
    in_maps = prep_inputs(inputs, plan)
    results = runner(in_maps)
    npc, n = plan.npc, plan.n
    out = np.empty((n, 4), np.float32)
    for c in range(NCORES):
        out[plan.perm[c * npc:(c + 1) * npc]] = results[c]["outT"].T
    return out


def kernel(**inputs) -> np.ndarray:
    return _run(inputs)



# revision 32
# speedup vs baseline: 1.4065x; 1.4065x over previous
"""DGCNN segmentation (3x EdgeConv max-aggregation + MLP head) on 8 Trainium2 cores.

Sharding: nodes are split into 8 equal contiguous blocks (one per core); each
core owns all edges whose *destination* lies in its block, so the scatter-max
aggregation is core-local.  Per-layer premultiplied node tables are computed
data-parallel over nodes and AllGather'd so every core can gather any source
node's contribution.

Per layer l (C_in -> C -> C, PyG EdgeConv):
    m_e   = relu(u'[dst_e] + v[src_e]) @ We_l          (per edge)
    h_i   = max_{e: dst_e = i} m_e + bb_l   (0 if no edges)
  where v = h @ Wa_l[C_in:]  (row table, gathered per edge),
        u' = h @ (Wa_l[:C_in] - Wa_l[C_in:]) + ba_l  (dst side, local bcast).

Device pipeline per core:
  - v row-tables in HBM (bf16); per-edge transposed gather via
    gpsimd.dma_gather(transpose=True) puts channels on partitions.  Gathers
    round-robin over 4 SWDGE queues so all four Q7 core-pairs generate
    descriptors in parallel (single queue leaves 3/4 of gpsimd idle).
  - Layer 1 needs no gather at all: the host uploads x rows in edge-slot
    order (layout-only prep), the first linear runs on the PE per tile.
  - DVE adds u'[dst] (broadcast over each node's slot run) to the gathered v,
    scalar-engine relu, PE matmul with We, segmented max over each node's
    padded slot-block via reduce_max, bias-add on eviction.
  - Edges are pre-sorted by destination and padded so each node owns a
    fixed-width slot run inside a 512-slot tile (identical tile structure on
    all 8 cores; only index data differs - the program is pure SPMD).
"""

import os
from dataclasses import dataclass, field

import numpy as np

import concourse.bass as bass
import concourse.mybir as mybir
import concourse.bacc as bacc
import concourse.tile as tile
from concourse import bass_utils, bass2jax
from concourse.bass import ds

F32 = mybir.dt.float32
BF16 = mybir.dt.bfloat16
I16 = mybir.dt.int16

NCORES = 8
TSLOT = 512          # edge-slots per tile (== max matmul moving free dim)
SUPER = 4            # tiles per dma_gather call
NQ = 4               # SWDGE queues used round-robin
AGP = 4              # AllGather pieces (table rows are piece-major)
EDGE_MAJOR = False   # slot order inside a tile: [d, n_t] vs [n_t, d]


# ----------------------------------------------------------------------------
# host-side preprocessing
# ----------------------------------------------------------------------------

@dataclass
class Plan:
    n: int
    npc: int
    tiles: list  # list of (D, n_t, pos0)  shared by all cores
    S: int       # total slots = TSLOT * len(tiles)
    perm: np.ndarray      # new position -> old node id
    vidx: list = field(default_factory=list)   # per-core wrapped [128, S/16] i16
    vsrc: list = field(default_factory=list)   # per-core flat [S] i64 (sentinel n)
    has_iso: bool = False  # any zero-degree node anywhere
    rpa: int = 0                               # local rows per AG piece
    psizes: list = field(default_factory=list)  # AG piece local row counts
    pbase: list = field(default_factory=list)   # piece base row in table


def make_plan(n: int, edge_index: np.ndarray) -> Plan:
    assert n % NCORES == 0
    npc = n // NCORES
    src = np.asarray(edge_index[0], dtype=np.int64)
    dst = np.asarray(edge_index[1], dtype=np.int64)
    deg = np.bincount(dst, minlength=n)

    # per-core block, degree-sorted (desc) within block
    perm = np.concatenate(
        [c * npc + np.argsort(-deg[c * npc:(c + 1) * npc], kind="stable")
         for c in range(NCORES)]
    )
    inv = np.empty(n, np.int64)
    inv[perm] = np.arange(n)
    src_n = inv[src]
    dst_n = inv[dst]
    deg_n = deg[perm]

    # shared tile structure from the max degree profile across cores
    degm = deg_n.reshape(NCORES, npc)
    maxdeg = degm.max(axis=0)
    tiles = []
    pos = 0
    while pos < npc:
        d = int(maxdeg[pos])
        d = max(2, d + (d & 1))          # even, >= 2
        n_t = min(TSLOT // d, npc - pos)
        tiles.append((d, n_t, pos))
        pos += n_t
    S = TSLOT * len(tiles)

    plan = Plan(n=n, npc=npc, tiles=tiles, S=S, perm=perm)
    plan.has_iso = bool((deg == 0).any())

    # AllGather piece geometry: tables are PIECE-MAJOR so each AG piece's
    # output is a contiguous slab: table row of node (c, r) with r in piece p
    # = pbase[p] + c*psizes[p] + (r - p*rpa);  sentinel row stays at n.
    rpa = min(npc, (npc * 55 // 100 + 127) // 128 * 128)
    psizes = []
    r = 0
    while r < npc:
        psizes.append(min(rpa, npc - r))
        r += rpa
    pbase = [int(v) for v in np.concatenate([[0], np.cumsum(
        [NCORES * s for s in psizes])[:-1]])]
    plan.rpa, plan.psizes, plan.pbase = rpa, psizes, pbase

    def remap(idx):
        out = np.full_like(idx, n)
        real = idx < n
        c, r = idx[real] // npc, idx[real] % npc
        p = np.minimum(r // rpa, len(psizes) - 1)
        pb = np.asarray(pbase)[p]
        sz = np.asarray(psizes)[p]
        out[real] = pb + c * sz + (r - p * rpa)
        return out

    # per-core slot fill.  L2/L3 tiles are EDGE-MAJOR inside each tile:
    # slot(node nti, edge di) = tile_base + di * n_t + nti.  This makes the
    # broadcast u'-add 2x-mode eligible on DVE (innermost dim is the packed
    # node dim for both operands); the edge matmul reads back node-major
    # via a strided AP so the segmented reduce stays innermost-axis.
    order = np.argsort(dst_n, kind="stable")
    src_s = src_n[order]
    dst_s = dst_n[order]
    starts = np.searchsorted(dst_s, np.arange(n))       # per new-id start
    for c in range(NCORES):
        dloc = deg_n[c * npc:(c + 1) * npc]
        vfill = np.full(npc, n, np.int64)      # sentinel: zero row
        nz = dloc > 0
        gids = c * npc + np.arange(npc)
        vfill[nz] = src_s[starts[gids[nz]]]    # first in-edge's src

        vidx = np.full(S, n, np.int64)
        base_pos = np.empty(npc, np.int64)   # slot of edge-rank 0 per node
        stride_pos = np.empty(npc, np.int64)  # slot stride between ranks
        for ti, (d, n_t, pos0) in enumerate(tiles):
            sl0 = ti * TSLOT
            p = np.arange(pos0, pos0 + n_t)
            if EDGE_MAJOR:
                base_pos[p] = sl0 + (p - pos0)
                stride_pos[p] = n_t
                vidx[sl0:sl0 + n_t * d] = np.tile(vfill[p], d)
            else:
                base_pos[p] = sl0 + (p - pos0) * d
                stride_pos[p] = 1
                vidx[sl0:sl0 + n_t * d] = np.repeat(vfill[p], d)
        # overwrite real edges
        m = (dst_s >= c * npc) & (dst_s < (c + 1) * npc)
        es, ed = src_s[m], dst_s[m] - c * npc
        # rank within node: edges of a node are contiguous since sorted by dst
        rank = np.arange(len(ed)) - np.searchsorted(ed, ed)
        slots = base_pos[ed] + rank * stride_pos[ed]
        vidx[slots] = es

        w = remap(vidx).astype(np.int16).reshape(-1, 16).T   # [16, S/16]
        plan.vidx.append(np.tile(w, (8, 1)).copy())   # [128, S/16]
        plan.vsrc.append(vidx)
    return plan


def prep_inputs(inputs: dict, plan: Plan) -> list:
    """Build per-core in_maps (keys = dram tensor names)."""
    n, npc, perm = plan.n, plan.npc, plan.perm
    f32 = np.float32
    import ml_dtypes
    bf16 = ml_dtypes.bfloat16

    x = np.asarray(inputs["x"], f32)[perm]              # [n, 3] permuted
    deg = np.bincount(np.asarray(inputs["edge_index"][1]), minlength=n)
    mask = (deg[perm] > 0).astype(f32)                  # new order

    w1a = np.asarray(inputs["w1a"], f32); b1a = np.asarray(inputs["b1a"], f32)
    w1b = np.asarray(inputs["w1b"], f32); b1b = np.asarray(inputs["b1b"], f32)
    w2a = np.asarray(inputs["w2a"], f32); b2a = np.asarray(inputs["b2a"], f32)
    w2b = np.asarray(inputs["w2b"], f32); b2b = np.asarray(inputs["b2b"], f32)
    w3a = np.asarray(inputs["w3a"], f32); b3a = np.asarray(inputs["b3a"], f32)
    w3b = np.asarray(inputs["w3b"], f32); b3b = np.asarray(inputs["b3b"], f32)
    wm1 = np.asarray(inputs["wm1"], f32); bm1 = np.asarray(inputs["bm1"], f32)
    wm2 = np.asarray(inputs["wm2"], f32); bm2 = np.asarray(inputs["bm2"], f32)
    wm3 = np.asarray(inputs["wm3"], f32); bm3 = np.asarray(inputs["bm3"], f32)

    # per-layer split: Wd = Wa[:cin]-Wa[cin:] (dst side), Wb = Wa[cin:] (src)
    w1a6 = np.zeros((8, 64), f32); w1a6[:6] = w1a       # [8, 64] K-pad
    wd2 = w2a[:64] - w2a[64:]                           # [64, 128]
    wb2 = w2a[64:]                                      # [64, 128]
    wd3 = w3a[:128] - w3a[128:]                         # [128, 256]
    wb3 = w3a[128:]                                     # [128, 256]

    # bb-fold: hstack holds h0 = h - bb (the segmented max without the
    # post-aggregation bias), so the reduce can write hstack directly.
    # Since everything downstream of h is linear in h, bb folds into the
    # consumers' biases:  pre_{l+1} = [h_i, h_j - h_i] @ Wa + ba
    #   = h0_i@Wd + h0_j@Wb + (ba + bb@Wa[:cin]);  head: bm1 += bbrow@wm1.
    # Only valid when no node is isolated (true h = h0 + bb for every node).
    if not plan.has_iso:
        b2a = b2a + b1b @ w2a[:64]
        b3a = b3a + b2b @ w3a[:128]
        bbrow = np.zeros(512, f32)
        bbrow[0:64] = b1b
        bbrow[128:256] = b2b
        bbrow[256:512] = b3b

    # second-linear (edge matmul) weights, bf16 lhsT layout
    we1 = w1b.astype(f32)                               # [64, 64]
    we2 = w2b.astype(f32)                               # [128, 128]
    we3 = np.ascontiguousarray(
        w3b.reshape(2, 128, 256).transpose(1, 0, 2))    # [128, k, 256]

    ba1 = b1a.reshape(64, 1).astype(f32)
    ba2 = b2a.reshape(128, 1).astype(f32)
    ba3 = b3a.reshape(2, 128).T.astype(f32)             # [128, 2]
    # post-aggregation bias (applied on eviction; bbm variant masks isolated)
    bb1 = b1b.reshape(64, 1).astype(f32)
    bb2 = b2b.reshape(128, 1).astype(f32)
    bb3 = b3b.reshape(2, 128).T.astype(f32)             # [128, 2]

    # bmask_l [C, J, npc] = bb * mask  (only used when isolated nodes exist)
    def bmask(bb_cj, mloc):
        # bb_cj: [C, J] f32 -> [C, J, npc] bf16 = bb * mask
        import ml_dtypes
        out = bb_cj[:, :, None] * mloc[None, None, :]
        return np.ascontiguousarray(out, dtype=ml_dtypes.bfloat16)

    # head weights: rearrange wm1 rows to hstack layout [h1(64) 0(64) h2 h3]
    wm1_arr = np.zeros((512, 512), f32)
    wm1_arr[0:64] = wm1[0:64]
    wm1_arr[128:256] = wm1[64:192]
    wm1_arr[256:512] = wm1[192:448]
    if not plan.has_iso:
        bm1 = bm1 + bbrow @ wm1_arr

    # edge features [x_dst, x_src - x_dst] in slot order (channel-major,
    # 8-padded); raw-input prep only, no model weights involved
    xpad = np.zeros((n + 1, 3), f32)
    xpad[:n] = x

    # per-slot dst node ids (from the fixed tile structure)
    dstid = np.empty(plan.S, np.int64)
    for ti, (d, n_t, pos0) in enumerate(plan.tiles):
        sl0 = ti * TSLOT
        p = np.arange(pos0, pos0 + n_t)
        dstid[sl0:sl0 + n_t * d] = (np.tile(p, d) if EDGE_MAJOR
                                    else np.repeat(p, d))
        dstid[sl0 + n_t * d:sl0 + TSLOT] = pos0

    in_maps = []
    for c in range(NCORES):
        mloc = mask[c * npc:(c + 1) * npc]
        xs = xpad[plan.vsrc[c]]                             # [S, 3] src rows
        xd = x[c * npc:(c + 1) * npc][dstid]                # [S, 3] dst rows
        xe = np.zeros((8, plan.S), f32)
        xe[0:3] = xd.T
        xe[3:6] = (xs - xd).T
        # sentinel slots (isolated nodes): zero feature
        sent = plan.vsrc[c] >= n
        xe[:, sent] = 0.0
        m = {
            "xe": np.ascontiguousarray(xe).astype(bf16),
            "vidx": plan.vidx[c],
            "w1a6": w1a6.astype(bf16),
            "wd2": wd2.astype(bf16), "wd3": wd3.astype(bf16),
            "wb2": wb2.astype(bf16), "wb3": wb3.astype(bf16),
            "we1": we1.astype(bf16), "we2": we2.astype(bf16),
            "we3": we3.astype(bf16),
            "ba1": ba1, "ba2": ba2, "ba3": ba3,
            "bb1": bb1, "bb2": bb2, "bb3": bb3,
            "bm1": bmask(bb1, mloc), "bm2": bmask(bb2, mloc),
            "bm3": bmask(bb3, mloc),
            "wh1": np.ascontiguousarray(
                wm1_arr.reshape(4, 128, 512).transpose(1, 0, 2)).astype(bf16),
            "wh2": np.ascontiguousarray(
                wm2.reshape(4, 128, 256).transpose(1, 0, 2)).astype(bf16),
            "wh3": np.ascontiguousarray(
                wm3.reshape(2, 128, 4).transpose(1, 0, 2)).astype(bf16),
            "bh1": np.ascontiguousarray(bm1.reshape(4, 128).T),
            "bh2": np.ascontiguousarray(bm2.reshape(2, 128).T),
            "bh3": bm3.reshape(4, 1).astype(f32),
        }
        in_maps.append(m)
    return in_maps


# ----------------------------------------------------------------------------
# device program
# ----------------------------------------------------------------------------

LAYERS = [
    # cin: input chans, cmid: relu width, cout: out chans, J: 128-chunks of
    # cmid/cout, src_chunk: hstack chunk range of the input
    dict(name="1", cin=3, cmid=64, cout=64, J=1, out_chunks=[0]),
    dict(name="2", cin=64, cmid=128, cout=128, J=1, out_chunks=[1]),
    dict(name="3", cin=128, cmid=256, cout=256, J=2, out_chunks=[2, 3]),
]


def build_program(plan: Plan, nlayers: int = 3, with_head: bool = True,
                  with_edge: bool = True, reps: int = 1):
    assert reps == 1 or (nlayers == 3 and with_head and with_edge)
    n, npc, S = plan.n, plan.npc, plan.S
    ntiles = len(plan.tiles)
    nc = bacc.Bacc(
        "TRN2", target_bir_lowering=False, debug=False,
        enable_asserts=False, num_devices=NCORES, num_swdge_queues=NQ,
    )
    RG = [list(range(NCORES))]

    # ---- dram tensors -------------------------------------------------------
    din = {}
    def dram_in(name, shape, dt):
        din[name] = nc.dram_tensor(name, list(shape), dt, kind="ExternalInput")
        return din[name]

    xe_d = dram_in("xe", (8, S), BF16)
    vidx_d = dram_in("vidx", (128, S // 16), I16)
    w1a6_d = dram_in("w1a6", (8, 64), BF16)
    wd_d = [None, dram_in("wd2", (64, 128), BF16),
            dram_in("wd3", (128, 256), BF16)]
    wb_d = [None, dram_in("wb2", (64, 128), BF16),
            dram_in("wb3", (128, 256), BF16)]
    we_d = [dram_in("we1", (64, 64), BF16), dram_in("we2", (128, 128), BF16),
            dram_in("we3", (128, 2, 256), BF16)]
    ba_d = [dram_in("ba1", (64, 1), F32), dram_in("ba2", (128, 1), F32),
            dram_in("ba3", (128, 2), F32)]
    bb_d = [dram_in("bb1", (64, 1), F32), dram_in("bb2", (128, 1), F32),
            dram_in("bb3", (128, 2), F32)]
    if plan.has_iso:
        bm_d = [dram_in("bm1", (64, 1, npc), BF16),
                dram_in("bm2", (128, 1, npc), BF16),
                dram_in("bm3", (128, 2, npc), BF16)]
    wh_d = [dram_in("wh1", (128, 4, 512), BF16),
            dram_in("wh2", (128, 4, 256), BF16),
            dram_in("wh3", (128, 2, 4), BF16)]
    bh_d = [dram_in("bh1", (128, 4), F32), dram_in("bh2", (128, 2), F32),
            dram_in("bh3", (4, 1), F32)]
    outT = nc.dram_tensor("outT", [4, npc], F32, kind="ExternalOutput")

    # Tables for layers 2,3.  The AllGather moves raw h (cin channels,
    # channel-major slabs — half the bytes of premultiplied v); each core
    # then computes v = h @ Wb for ALL nodes locally and writes the
    # row-major gather table vtab.  dma_gather cannot read Shared-addr-
    # space scratchpad, so everything stays Local.
    vcols = [None, 128, 256]          # v width per consuming layer
    hcols = [None, 64, 128]           # h (AG payload) width
    nbuf = min(2, reps)               # table double-buffering by rep parity
    vtab_t = [[None] + [nc.dram_tensor(f"vtab{i}_{b}",
                                       [n + 1, vcols[i - 1]],
                                       BF16, kind="Internal")
                        for i in (2, 3)] for b in range(nbuf)]
    hagp_t = [[None] for _ in range(nbuf)]
    tabTp_t = [[None] for _ in range(nbuf)]
    for b in range(nbuf):
        for i in (2, 3):
            hagp_t[b].append([nc.dram_tensor(
                f"hag{i}_{p}_{b}", [hcols[i - 1], plan.psizes[p]], BF16,
                kind="Internal") for p in range(len(plan.psizes))])
            tabTp_t[b].append([nc.dram_tensor(
                f"tabT{i}_{p}_{b}",
                [NCORES * hcols[i - 1], plan.psizes[p]], BF16,
                kind="Internal") for p in range(len(plan.psizes))])

    with tile.TileContext(nc) as tc:
        with (
            tc.tile_pool(name="singles", bufs=1) as sing,
            tc.tile_pool(name="stage", bufs=2) as stg,
            tc.tile_pool(name="xe", bufs=2) as xep,
            tc.tile_pool(name="gat2", bufs=4) as gat2,
            tc.tile_pool(name="gat3", bufs=4) as gat3,
            tc.tile_pool(name="edge", bufs=3) as edg,
            tc.tile_pool(name="vtj2", bufs=2) as vtj2,
            tc.tile_pool(name="vtj3", bufs=3) as vtj3,
            tc.tile_pool(name="hst", bufs=2) as hsp,
            tc.tile_pool(name="us", bufs=1) as usp,
        ):
            vtj = [None, vtj2, vtj3]
            # ---- load constants into SBUF ----
            def load(dt_handle, shape, dtype, tag):
                t = sing.tile(list(shape), dtype, tag=tag)
                nc.sync.dma_start(t, dt_handle[...])
                return t

            vidx_s = load(vidx_d, (128, S // 16), I16, "vidx")
            w1a6_s = load(w1a6_d, (8, 64), BF16, "w1a6")
            wd_s = [None] + [load(wd_d[i], wd_d[i].shape, BF16, f"wd{i}")
                             for i in (1, 2)]
            wb_s = [None] + [load(wb_d[i], wb_d[i].shape, BF16, f"wb{i}")
                             for i in (1, 2)]
            we_s = [load(we_d[i], we_d[i].shape, BF16, f"we{i}")
                    for i in range(3)]
            ba_s = [load(ba_d[i], ba_d[i].shape, F32, f"ba{i}")
                    for i in range(3)]
            bb_s = [load(bb_d[i], bb_d[i].shape, F32, f"bb{i}")
                    for i in range(3)]
            bm_s = ([load(bm_d[i], bm_d[i].shape, BF16, f"bm{i}")
                     for i in range(3)] if plan.has_iso else None)
            wh_s = [load(wh_d[i], wh_d[i].shape, BF16, f"wh{i}")
                    for i in range(3)]
            bh_s = [load(bh_d[i], bh_d[i].shape, F32, f"bh{i}")
                    for i in range(3)]

            zrow = sing.tile([1, 512], BF16)
            nc.vector.memset(zrow, 0.0)

            # hstack [h1;0 | h2 | h3a | h3b], u' tables and the output
            # staging buffer are double-buffered by rep parity so rep i+1's
            # L1 phase can overlap rep i's head (WAR would otherwise
            # serialize the unrolled benchmark pipeline)
            hstack = None
            u_s = None

            # ---- table builders -------------------------------------------
            # Layer l's u'/v tables are built from layer l-1's output, whose
            # node aggregations complete progressively (tiles are position-
            # ordered).  Builders are *emitted* interleaved with the producing
            # layer's edge loop so the PE executes them as soon as the data is
            # ready, and the AllGather runs in AGP staggered pieces that
            # overlap the edge phase instead of serializing after it.
            stagger = npc // 16

            def hsrc(li):
                return hstack[0:64, 0, :] if li == 1 else hstack[:, 1, :]

            def u_builder(li, pool):
                L = LAYERS[li]
                cpj = L["cmid"] // L["J"]
                J = L["J"]
                hprev = hsrc(li)
                wdl = wd_s[li]
                state = dict(c=0)
                UW = 256

                def emit(ready):
                    while state["c"] < npc:
                        c0 = state["c"]
                        w = min(UW, npc - c0)
                        if c0 + w > ready:
                            return
                        pu = pool.tile([128, 2, UW], F32, tag="ub",
                                       name="pu", bufs=1)
                        for jj in range(J):
                            nc.tensor.matmul(pu[0:cpj, jj, 0:w],
                                             wdl[:, ds(jj * 128, cpj)],
                                             hprev[:, ds(c0, w)],
                                             start=True, stop=True)
                            nc.scalar.activation(
                                u_s[li][0:cpj, jj, ds(c0, w)],
                                pu[0:cpj, jj, 0:w],
                                mybir.ActivationFunctionType.Identity,
                                bias=ba_s[li][:, jj:jj + 1])
                        state["c"] = c0 + w
                return emit

            def v_builder(li, pool, tb):
                cv = LAYERS[li]["cmid"]          # v table width (== cout)
                cin = LAYERS[li]["cin"]          # AG payload channels
                kch = 512 // cv                  # node chunks per PSUM bank
                hprev = hsrc(li)
                tjp = vtj[li]
                vtab = vtab_t[tb][li]
                hagp = hagp_t[tb][li]
                tabTp = tabTp_t[tb][li]
                state = dict(a=0)
                rpa = plan.rpa
                nc.sync.dma_start(vtab[n:n + 1, :], zrow[:, 0:cv])

                def emit(ready, flush=False):
                    while state["a"] < len(plan.psizes):
                        p = state["a"]
                        a0 = p * rpa
                        rows = plan.psizes[p]
                        if a0 + rows > ready:
                            break
                        if not flush and a0 + rows + stagger > ready:
                            break
                        # stage local h slab (channel-major) and AllGather
                        nc.sync.dma_start(hagp[p][...],
                                          hprev[0:cin, ds(a0, rows)])
                        nc.gpsimd.collective_compute(
                            "AllGather", mybir.AluOpType.bypass, RG,
                            ins=[hagp[p][...]],
                            outs=[tabTp[p][...]],
                        )
                        # v rows for every core's block of this piece
                        for c in range(NCORES):
                            # per-core sub-slab of the gathered table
                            tj = tjp.tile([128, rpa], BF16,
                                          tag=f"tj{li}", name="tj")
                            nc.sync.dma_start(
                                tj[0:cin, 0:rows],
                                tabTp[p][ds(c * cin, cin), :])
                            base = plan.pbase[p] + c * rows
                            j0 = 0
                            while j0 < rows:
                                g = min(kch * 128, rows - j0)
                                nk = (g + 127) // 128
                                pv = pool.tile([128, kch, cv], F32,
                                               tag="vrow", name="pv",
                                               bufs=1)
                                for k in range(nk):
                                    m = min(128, g - k * 128)
                                    nc.tensor.matmul(
                                        pv[0:m, k, 0:cv],
                                        tj[0:cin, ds(j0 + k * 128, m)],
                                        wb_s[li][...],
                                        start=True, stop=True)
                                st = stg.tile([128, kch, cv], BF16,
                                              tag="uv_stage")
                                eng = nc.scalar if c % 2 == 0 else nc.vector
                                if g == kch * 128:
                                    if eng is nc.scalar:
                                        eng.copy(st[...], pv[...])
                                    else:
                                        eng.tensor_copy(st[...], pv[...])
                                    nc.sync.dma_start(
                                        vtab[ds(base + j0, g), :]
                                        .rearrange("(k p) v -> p k v",
                                                   p=128),
                                        st[...])
                                else:
                                    for k in range(nk):
                                        m = min(128, g - k * 128)
                                        if eng is nc.scalar:
                                            eng.copy(st[0:m, k, 0:cv],
                                                     pv[0:m, k, 0:cv])
                                        else:
                                            eng.tensor_copy(
                                                st[0:m, k, 0:cv],
                                                pv[0:m, k, 0:cv])
                                        nc.sync.dma_start(
                                            vtab[
                                                ds(base + j0 + k * 128,
                                                   m), :],
                                            st[0:m, k, 0:cv])
                                j0 += g
                        state["a"] += 1
                return emit

            # ---- head builder (interleaved into L3's edge phase) ---------
            HW = 256

            def head_builder(pool):
                state = dict(c=0)

                def emit(ready, flush=False):
                    while state["c"] < npc:
                        c0 = state["c"]
                        w = min(HW, npc - c0)
                        if not flush and c0 + w + stagger > ready:
                            break
                        ps1 = pool.tile([128, 4, HW], F32, tag="h_ps1",
                                        name="ps1", bufs=1)
                        for jj in range(4):
                            for kk in range(4):
                                nc.tensor.matmul(
                                    ps1[:, jj, 0:w],
                                    wh_s[0][:, kk, ds(jj * 128, 128)],
                                    hstack[:, kk, ds(c0, w)],
                                    start=(kk == 0), stop=(kk == 3))
                        m1 = edg.tile([128, 4, HW], BF16, tag="h_m1")
                        for jj in range(4):
                            nc.scalar.activation(
                                m1[:, jj, 0:w], ps1[:, jj, 0:w],
                                mybir.ActivationFunctionType.Relu,
                                bias=bh_s[0][:, jj:jj + 1])
                        ps2 = pool.tile([128, 2, HW], F32, tag="h_ps2",
                                        name="ps2", bufs=1)
                        for jj in range(2):
                            for kk in range(4):
                                nc.tensor.matmul(
                                    ps2[:, jj, 0:w],
                                    wh_s[1][:, kk, ds(jj * 128, 128)],
                                    m1[:, kk, 0:w],
                                    start=(kk == 0), stop=(kk == 3))
                        m2 = edg.tile([128, 2, HW], BF16, tag="h_m2")
                        for jj in range(2):
                            nc.scalar.activation(
                                m2[:, jj, 0:w], ps2[:, jj, 0:w],
                                mybir.ActivationFunctionType.Relu,
                                bias=bh_s[1][:, jj:jj + 1])
                        ps3 = pool.tile([4, HW], F32, tag="h_ps3",
                                        name="ps3", bufs=1)
                        for kk in range(2):
                            nc.tensor.matmul(ps3[:, 0:w], wh_s[2][:, kk, :],
                                             m2[:, kk, 0:w],
                                             start=(kk == 0), stop=(kk == 1))
                        oc_t = stg.tile([4, HW], F32, tag="oc")
                        nc.scalar.activation(
                            oc_t[:, 0:w], ps3[:, 0:w],
                            mybir.ActivationFunctionType.Identity,
                            bias=bh_s[2][:, 0:1])
                        nc.sync.dma_start(outT[:, ds(c0, w)], oc_t[:, 0:w])
                        state["c"] = c0 + w
                return emit

            # =============== per layer ===============
            # reps > 1 unrolls the whole computation back-to-back for
            # device-time benchmarking (amortizes per-dispatch overhead)
            from contextlib import ExitStack
            for rep, (li, L) in ((r, x) for r in range(reps)
                                 for x in enumerate(LAYERS[:nlayers])):
                cin, cmid, cout, J = L["cin"], L["cmid"], L["cout"], L["J"]
                if li == 0:
                    hstack = hsp.tile([128, 4, npc], BF16, tag="hstack",
                                      name="hstack")
                    u_s = [None,
                           usp.tile([128, 1, npc], BF16, tag="u2",
                                    name="u2"),
                           usp.tile([128, 2, npc], BF16, tag="u3",
                                    name="u3")]
                    if rep < 2:
                        # rows 64:128 of chunk 0 are never written by L1
                        # but are read by the head (against zero weights);
                        # keep them finite.  Later reps reuse finite data.
                        nc.vector.memset(
                            hstack[64:128, 0, :] if with_edge
                            else hstack[...], 0.0)
                with ExitStack() as phase:
                    psp = phase.enter_context(tc.tile_pool(
                        name=f"ps{rep}_{li}", bufs=2, space="PSUM"))
                    # builders for the NEXT layer run inside this edge phase
                    nxt_u = (u_builder(li + 1, psp)
                             if li + 1 < nlayers else None)
                    nxt_v = (v_builder(li + 1, psp, rep % nbuf)
                             if li + 1 < nlayers else None)
                    head = (head_builder(psp)
                            if (with_head and li == nlayers - 1) else None)

                    # ---- edge phase ----
                    ebufs = 3 if li == 0 else (4 if li == 1 else 2)
                    for sti, st0 in enumerate(range(0, ntiles, SUPER)
                                              if with_edge else []):
                        g = min(SUPER, ntiles - st0)
                        nidx = g * TSLOT
                        if li == 0:
                            xet = xep.tile([8, SUPER * TSLOT], BF16, tag="xe")
                            nc.sync.dma_start(
                                xet[:, 0:nidx],
                                xe_d[:, ds(st0 * TSLOT, nidx)])
                        else:
                            pool = gat2 if li == 1 else gat3
                            vg = pool.tile([128, J, nidx], BF16,
                                           tag=f"vg{li}", name=f"vg{li}")
                            c0 = st0 * TSLOT // 16
                            nc.gpsimd.dma_gather(
                                vg[...], vtab_t[rep % nbuf][li][...],
                                vidx_s[:, ds(c0, nidx // 16)],
                                nidx, nidx, elem_size=cout, transpose=True,
                                single_packet=False, queue_num=sti % NQ)
                        for tt in range(g):
                            d, n_t, pos0 = plan.tiles[st0 + tt]
                            T = n_t * d
                            o = tt * TSLOT
                            if li == 0:
                                # pre = W1a^T [x_i, x_j-x_i]  (host-built xe)
                                pp = psp.tile([64, 1, TSLOT], F32,
                                              tag="pre_ps", name="pp", bufs=3)
                                nc.tensor.matmul(pp[:, 0, 0:T], w1a6_s,
                                                 xet[:, ds(o, T)],
                                                 start=True, stop=True)
                                rl = edg.tile([64, 1, TSLOT], BF16, tag="rl1")
                                nc.scalar.activation(
                                    rl[:, 0, 0:T], pp[:, 0, 0:T],
                                    mybir.ActivationFunctionType.Relu,
                                    bias=ba_s[0][:, 0:1])
                            else:
                                if EDGE_MAJOR:
                                    # slots [d, n_t], node dim innermost:
                                    # both add operands 2-byte packed (2x)
                                    ub = u_s[li][:, :, ds(pos0, n_t)
                                                 ].rearrange(
                                        "p j (one nt) -> p j one nt",
                                        one=1).broadcast_to(
                                        (128, J, d, n_t))
                                    vgv = vg[:, :, ds(o, T)].rearrange(
                                        "p j (d nt) -> p j d nt", nt=n_t)
                                else:
                                    ub = u_s[li][:, :, ds(pos0, n_t)
                                                 ].rearrange(
                                        "p j (nt one) -> p j nt one",
                                        one=1).broadcast_to(
                                        (128, J, n_t, d))
                                    vgv = vg[:, :, ds(o, T)].rearrange(
                                        "p j (nt d) -> p j nt d", d=d)
                                pre = edg.tile([128, J, TSLOT], BF16,
                                               tag=f"pre{li}", name="pre")
                                nc.vector.tensor_add(
                                    pre[:, :, 0:T].rearrange(
                                        "p j (a b) -> p j a b",
                                        b=n_t if EDGE_MAJOR else d),
                                    vgv, ub)
                                rl = edg.tile([128, J, TSLOT], BF16,
                                              tag=f"rl{li}")
                                nc.scalar.activation(
                                    rl[:, :, 0:T], pre[:, :, 0:T],
                                    mybir.ActivationFunctionType.Relu)
                            # m = rl @ We  (K-accumulate over J chunks).
                            # In edge-major mode the rhs reads rl back in
                            # node-major order so PSUM comes out node-major
                            # and the segmented max reduces innermost.
                            ps = psp.tile([128, J, TSLOT], F32,
                                          tag="edge_ps", name="ps",
                                          bufs=ebufs)
                            for jj in range(J):
                                for kk in range(J):
                                    w = (we_s[li][0:cmid, :] if J == 1
                                         else we_s[li][:, kk,
                                                       ds(jj * 128, 128)])
                                    rhs = rl[:, kk, 0:T]
                                    if EDGE_MAJOR:
                                        rhs = rhs.rearrange(
                                            "p (d nt) -> p nt d", nt=n_t)
                                    nc.tensor.matmul(
                                        ps[0:cout // J, jj, 0:T], w, rhs,
                                        start=(kk == 0), stop=(kk == J - 1))
                            oc = L["out_chunks"]
                            if plan.has_iso:
                                # segmented max + bias*mask add
                                tmp = edg.tile([128, 2, 256], BF16, tag="agg")
                                nc.vector.reduce_max(
                                    tmp[0:cout // J, 0:J, 0:n_t],
                                    ps[0:cout // J, 0:J, 0:T].rearrange(
                                        "p j (nt d) -> p j nt d", d=d),
                                    axis=mybir.AxisListType.X)
                                nc.vector.tensor_add(
                                    hstack[0:cout // J, oc[0]:oc[0] + J,
                                           ds(pos0, n_t)],
                                    tmp[0:cout // J, 0:J, 0:n_t],
                                    bm_s[li][:, :, ds(pos0, n_t)])
                            else:
                                # bb is folded downstream: segmented max
                                # writes hstack directly
                                nc.vector.reduce_max(
                                    hstack[0:cout // J, oc[0]:oc[0] + J,
                                           ds(pos0, n_t)],
                                    ps[0:cout // J, 0:J, 0:T].rearrange(
                                        "p j (nt d) -> p j nt d", d=d),
                                    axis=mybir.AxisListType.X)
                        # emit next-layer table chunks whose inputs are done
                        last = plan.tiles[st0 + g - 1]
                        ready = last[2] + last[1]
                        if nxt_u is not None:
                            nxt_u(ready)
                            nxt_v(ready)
                        if head is not None:
                            head(ready)
                    if nxt_u is not None:
                        nxt_u(npc)
                        nxt_v(npc, flush=True)
                    if head is not None:
                        head(npc, flush=True)

            # =============== head (debug fallback paths) ===============
            if not with_head:
                with tc.tile_pool(name="psum1", bufs=1, space="PSUM"):
                    dbg = sing.tile([4, npc], F32, tag="dbg")
                    nc.vector.tensor_copy(dbg, hstack[0:4, 0, :])
                    nc.sync.dma_start(outT[...], dbg)
            elif nlayers < 3:
                with tc.tile_pool(name="psum1", bufs=1, space="PSUM") as psp1:
                    hb = head_builder(psp1)
                    hb(npc, flush=True)

    nc.compile()
    return nc


# ----------------------------------------------------------------------------
# entry point
# ----------------------------------------------------------------------------

class Runner:
    """Compile once; keep the jitted sharded executable for cheap re-runs."""

    def __init__(self, nc):
        import jax
        from jax.sharding import Mesh, PartitionSpec
        from jax.experimental.shard_map import shard_map

        bass2jax.install_neuronx_cc_hook()
        self.nc = nc
        partition_name = (nc.partition_id_tensor.name
                          if nc.partition_id_tensor else None)
        in_names, out_names, out_avals, zero_outs = [], [], [], []
        for alloc in nc.m.functions[0].allocations:
            if not isinstance(alloc, mybir.MemoryLocationSet):
                continue
            name = alloc.memorylocations[0].name
            if alloc.kind == "ExternalInput":
                if name != partition_name:
                    in_names.append(name)
            elif alloc.kind == "ExternalOutput":
                shape = tuple(alloc.tensor_shape)
                dtype = mybir.dt.np(alloc.dtype)
                out_names.append(name)
                out_avals.append(jax.core.ShapedArray(shape, dtype))
                zero_outs.append(np.zeros(shape, dtype))
        n_params = len(in_names)
        all_in = in_names + out_names
        if partition_name is not None:
            all_in.append(partition_name)
        self.in_names = in_names
        self.out_names = out_names
        self.out_avals = out_avals
        self.zero_outs = zero_outs

        def _body(*args):
            operands = list(args)
            if partition_name is not None:
                operands.append(bass2jax.partition_id_tensor())
            return tuple(bass2jax._bass_exec_p.bind(
                *operands, out_avals=tuple(out_avals),
                in_names=tuple(all_in), out_names=tuple(out_names),
                lowering_input_output_aliases=(),
                sim_require_finite=True, sim_require_nnan=True, nc=nc))

        devices = jax.devices()[:NCORES]
        mesh = Mesh(np.asarray(devices), ("core",))
        self.mesh = mesh
        n_outs = len(out_names)
        self.sharded = jax.jit(
            shard_map(_body, mesh=mesh,
                      in_specs=(PartitionSpec("core"),) * (n_params + n_outs),
                      out_specs=(PartitionSpec("core"),) * n_outs,
                      check_rep=False),
            keep_unused=True)

    def prepare(self, in_maps):
        """Upload per-core inputs to the devices once; returns dev args."""
        import jax
        from jax.sharding import NamedSharding, PartitionSpec
        sh = NamedSharding(self.mesh, PartitionSpec("core"))
        concat_in = [
            np.concatenate([np.asarray(in_maps[c][nm])
                            for c in range(NCORES)], axis=0)
            for nm in self.in_names]
        concat_zeros = [np.zeros((NCORES * z.shape[0], *z.shape[1:]), z.dtype)
                        for z in self.zero_outs]
        args = [jax.device_put(a, sh) for a in concat_in + concat_zeros]
        jax.block_until_ready(args)
        return args

    def run(self, dev_args):
        import jax
        outs = self.sharded(*dev_args)
        jax.block_until_ready(outs)
        return outs

    def __call__(self, in_maps):
        outs = self.run(self.prepare(in_maps))
        return [
            {nm: np.asarray(outs[i]).reshape(
                NCORES, *self.out_avals[i].shape)[c]
             for i, nm in enumerate(self.out_names)}
            for c in range(NCORES)]


_CACHE = {}


def get_compiled(inputs: dict):
    x = np.asarray(inputs["x"])
    ei = np.asarray(inputs["edge_index"])
    n = x.shape[0]
    key = (n, ei.shape[1], hash(ei.tobytes()))
    if key not in _CACHE:
        plan = make_plan(n, ei)
        nc = build_program(plan)
        _CACHE.clear()
        _CACHE[key] = (plan, Runner(nc))
    return _CACHE[key]


def _run(inputs: dict) -> np.ndarray:
    plan, runner = get_compiled(inputs)
    in_maps = prep_inputs(inputs, plan)
    results = runner(in_maps)
    npc, n = plan.npc, plan.n
    out = np.empty((n, 4), np.float32)
    for c in range(NCORES):
        out[plan.perm[c * npc:(c + 1) * npc]] = results[c]["outT"].T
    return out


def kernel(**inputs) -> np.ndarray:
    return _run(inputs)

